# revision 28
# baseline (speedup 1.0000x reference)
"""Trainium2 Bass kernel for nn_ComputeVecLoss (vector loss over keypoint graphs).

Math (per batch b):
  For every keypoint pair (i>j) sample 5 points on the segment; cdis = mean
  over the 5 points of the min squared distance to the 4096 gt points; an edge
  exists when cdis < 1e-3.  Loss = sum over edges of |u_i.u_j| / (|u_i||u_j|)
  divided by (1 + edge count), u_k = p0 - p_k.

Kernel structure (per core, 2 batches):
  * 850 unique query points (2 x (17 endpoints + 136*3 interiors)) are packed
    into 7 row tiles of 128.  d^2(K, g) = |K|^2 - 2 K.g + |g|^2 is ONE matmul
    row per (query, gt) pair: contraction dim 7 = per-batch [-2Kx, -2Ky, 1]
    blocks plus a shared [|K|^2] row that pairs with a ones-row in gaug, so
    PSUM holds finished squared distances (no post-min fixups).
  * min over the 4096 gt points: tensor_tensor_reduce min(in0,in1) with a
    chained per-partition accumulator -> pmin[128, 7] in SBUF directly.
  * Row order is chosen so the tail needs no DRAM gather: tiles 1-3 hold the
    3 interior samples of pairs 0-127 at partition = pair, tiles 4-6 of pairs
    128-255; tile 0 (run FIRST) holds the 34 endpoints plus the 16 leftover
    pairs' interiors.  cdis5 per pair is then a free-dim reduce plus one
    0/1-matrix matmul that also gathers endpoint u-features.
"""

import os
import sys

for _p in ("/opt/trn_rl_repo",):
    if os.path.isdir(_p) and _p not in sys.path:
        sys.path.append(_p)

import numpy as np

B, N, D = 16, 17, 2
M = 4096
COUNT = 5
MAXDIS = 1e-3
EPS_ABS = 1e-5
N_CORES = 8
BPC = B // N_CORES          # batches per core
NPAIR = N * (N - 1) // 2    # 136
PAIR2 = BPC * NPAIR         # 272 pairs per core
RPAD = 7 * 128              # 896 padded query rows per core
GROUPS = [(0, 128), (128, 128), (256, 16)]

PAIRS = [(i, j) for i in range(1, N) for j in range(i)]

# number of 512-col moving chunks per matmul instruction (1 => 512-col MMs)
MMW = 512


def _row_query(c):
    """Row index -> query descriptor ('end', b, k) or ('int', b, pair, k)."""
    t, p = divmod(c, 128)
    if t == 0:
        if p < 17:
            return ("end", 0, p)
        if p < 34:
            return ("end", 1, p - 17)
        if p < 82:
            i2, k = divmod(p - 34, 3)
            return ("int", 1, 120 + i2, k)
        return None
    if t <= 3:
        P, k = p, t - 1
    else:
        P, k = 128 + p, t - 4
    b, pr = divmod(P, NPAIR)
    return ("int", b, pr, k)


def _constants():
    # ct rows 0..33: coord map (batch-block); rows 34..35: per-batch 0/1
    # "query c belongs to batch b" masks (DMA'd into kaugT rows 3 and 6).
    ct = np.zeros((2 * N + 2, RPAD), np.float32)
    s12 = np.zeros((128, 2 * PAIR2), np.float32)
    wt = np.zeros((2 * N, 2 * N), np.float32)

    for c in range(RPAD):
        q = _row_query(c)
        if q is None:
            # unused rows: treat as batch-0 query at K=(0,0) so the row's
            # d^2 = |g|^2 >= 0 (keeps the exp/softmin path finite; the
            # tail multiplies these rows by 0).
            ct[2 * N + 0, c] = 1.0
            continue
        if q[0] == "end":
            _, b, k = q
            ct[17 * b + k, c] = -2.0
            ct[2 * N + b, c] = 1.0
        else:
            _, b, pr, k = q
            i, j = PAIRS[pr]
            tv = 0.25 * (k + 1)
            ct[17 * b + i, c] = -2.0 * tv
            ct[17 * b + j, c] = -2.0 * (1.0 - tv)
            ct[2 * N + b, c] = 1.0

    for P in range(PAIR2):
        b, pr = divmod(P, NPAIR)
        i, j = PAIRS[pr]
        s12[17 * b + i, P] = 1.0
        s12[17 * b + j, PAIR2 + P] = 1.0
        if P >= 256:
            i2 = P - 256
            for k in range(3):
                s12[34 + 3 * i2 + k, P] = 1.0

    for b in range(BPC):
        for m in range(N):
            wt[N * b, N * b + m] += 1.0
            wt[N * b + m, N * b + m] -= 1.0
    return ct, s12, wt


_CONSTS = None
_COMPILED = None


def _get_consts():
    global _CONSTS
    if _CONSTS is None:
        _CONSTS = _constants()
    return _CONSTS


def _build():
    import concourse.bass as bass
    import concourse.bacc as bacc
    import concourse.tile as tile
    from concourse import mybir

    f32 = mybir.dt.float32
    f32r = mybir.dt.float32r
    Alu = mybir.AluOpType
    Act = mybir.ActivationFunctionType
    X = mybir.AxisListType.X
    A = 20000.0

    nc = bacc.Bacc("TRN2", target_bir_lowering=False, debug=False,
                   num_devices=N_CORES)

    recon = nc.dram_tensor("recon", [BPC, N, D], f32, kind="ExternalInput").ap()
    gt = nc.dram_tensor("gt", [BPC, M, D], f32, kind="ExternalInput").ap()
    ct_d = nc.dram_tensor("ct", [2 * (N + 1), RPAD], f32,
                          kind="ExternalInput").ap()
    s12_d = nc.dram_tensor("s12", [128, 2 * PAIR2], f32,
                           kind="ExternalInput").ap()
    wt_d = nc.dram_tensor("wt", [2 * N, 2 * N], f32, kind="ExternalInput").ap()
    out_d = nc.dram_tensor("out", [2], f32, kind="ExternalOutput").ap()

    with tile.TileContext(nc) as tc:
        with (
            tc.tile_pool(name="singles", bufs=1) as singles,
            tc.tile_pool(name="work", bufs=3) as work,
            tc.tile_pool(name="hot", bufs=2, space="PSUM") as psum,
            tc.tile_pool(name="dram", bufs=1, space="DRAM") as dram,
        ):
            CD = 3 * BPC + 1  # 7 contraction rows

            # ================= critical kaug chain (high priority) ==========
            # ct -> cast -> kaug MM -> sqk -> k2 -> kaugT row0.  p1aug rows
            # 34/35 are one-hot picks of ct's two batch-mask rows, so the MM
            # emits kaugT rows 3/6 (batch masks) directly.
            with tc.high_priority():
                ctf = singles.tile([2 * (N + 1), RPAD], f32)
                nc.sync.dma_start(out=ctf[:], in_=ct_d[:])

                p1f = singles.tile([2 * (N + 1), CD], f32)
                nc.vector.memset(p1f[:], 0.0)
                for b in range(BPC):
                    nc.scalar.dma_start(
                        out=p1f[N * b:N * b + N, 1 + 3 * b:3 + 3 * b],
                        in_=recon[b])
                # one-hot mask-row selectors, sourced from known 1.0s in ct
                nc.sync.dma_start(out=p1f[2 * N:2 * N + 1, 3:4],
                                    in_=ct_d[2 * N:2 * N + 1, 0:1])
                nc.sync.dma_start(out=p1f[2 * N + 1:2 * N + 2, 6:7],
                                    in_=ct_d[2 * N + 1:2 * N + 2, 17:18])
                p1aug = singles.tile([2 * (N + 1), CD], f32r)
                nc.vector.tensor_copy(out=p1aug[:], in_=p1f[:])

                ct_sb = singles.tile([2 * (N + 1), RPAD], f32r)
                kaugT = singles.tile([CD, RPAD], f32r)
                sqk = singles.tile([CD, RPAD], f32r)
                ones7 = singles.tile([CD, 1], f32r)
                onesf = singles.tile([CD, 1], f32)
                nc.vector.memset(onesf[:], 1.0)
                nc.vector.tensor_copy(out=ones7[:], in_=onesf[:])
                for c0 in range(0, RPAD, 512):
                    ce = min(c0 + 512, RPAD)
                    nc.vector.tensor_copy(out=ct_sb[:, c0:ce], in_=ctf[:, c0:ce])
                    kp = psum.tile([CD, ce - c0], f32, tag="hot", name="kp")
                    nc.tensor.matmul(kp[:], p1aug[:], ct_sb[:, c0:ce],
                                     start=True, stop=True)
                    nc.vector.tensor_copy(out=kaugT[:, c0:ce], in_=kp[:])
                    nc.scalar.activation(out=sqk[:, c0:ce], in_=kaugT[:, c0:ce],
                                         func=Act.Square)
                    k2p = psum.tile([1, ce - c0], f32, tag="hot", name="k2p")
                    nc.tensor.matmul(k2p[:], ones7[:], sqk[:, c0:ce],
                                     start=True, stop=True)
                    nc.vector.tensor_scalar(out=kaugT[0:1, c0:ce], in0=k2p[:],
                                            scalar1=0.25, scalar2=-0.25,
                                            op0=Alu.mult, op1=Alu.add)

            # ================= gaug path ====================================
            gtb = singles.tile([128, 2 * D * M // 128], f32)  # [128, 128]
            for b in range(BPC):
                nc.sync.dma_start(
                    out=gtb[:, 64 * b:64 * (b + 1)],
                    in_=gt[b].rearrange("(p k) d -> p (k d)", p=128))
            sq = work.tile([128, 128], f32)
            nc.vector.tensor_mul(sq[:], gtb[:], gtb[:])
            ones32 = singles.tile([128, M // 128], f32)
            nc.vector.memset(ones32[:], 1.0)
            gxyz = singles.tile([128, CD, M // 128], f32r)
            nc.vector.tensor_copy(out=gxyz[:, 0, :], in_=ones32[:])
            for b in range(BPC):
                c0 = b * 2 * M // 128  # 64
                nc.vector.tensor_copy(out=gxyz[:, 1 + 3 * b, :],
                                      in_=gtb[:, c0 + 0:c0 + 64:2])
                nc.vector.tensor_copy(out=gxyz[:, 2 + 3 * b, :],
                                      in_=gtb[:, c0 + 1:c0 + 64:2])
                nc.vector.tensor_add(gxyz[:, 3 + 3 * b, :],
                                     sq[:, c0 + 0:c0 + 64:2],
                                     sq[:, c0 + 1:c0 + 64:2])
            gscr = dram.tile([CD, M], f32r)
            nc.scalar.dma_start(out=gscr.rearrange("c (p k) -> p c k", p=128),
                                in_=gxyz[:])
            gaug = singles.tile([CD, M], f32r)
            nc.sync.dma_start(out=gaug[:], in_=gscr[:])

            # ================= u-features ===================================
            p1_both = singles.tile([2 * N, D], f32)
            nc.sync.dma_start(out=p1_both[:],
                              in_=recon.rearrange("b n d -> (b n) d"))
            wt_sb = singles.tile([2 * N, 2 * N], f32)
            nc.sync.dma_start(out=wt_sb[:], in_=wt_d[:])
            s12_sb = singles.tile([128, 2 * PAIR2], f32)
            nc.sync.dma_start(out=s12_sb[:], in_=s12_d[:])
            eps_sb = singles.tile([2 * N, 1], f32)
            nc.vector.memset(eps_sb[:], float(D * EPS_ABS))
            ones_sb = singles.tile([128, 1], f32)
            nc.vector.memset(ones_sb[:], 1.0)

            u_ps = psum.tile([2 * N, D], f32, tag="hot", name="u_ps")
            nc.tensor.matmul(u_ps[:], wt_sb[:], p1_both[:], start=True,
                             stop=True)
            F = singles.tile([128, 4], f32)
            nc.vector.memset(F[:, 0:3], 0.0)
            nc.vector.tensor_copy(out=F[0:2 * N, 0:2], in_=u_ps[:])
            usq = work.tile([2 * N, D], f32)
            nc.vector.tensor_mul(usq[:], F[0:2 * N, 0:2], F[0:2 * N, 0:2])
            ua = work.tile([2 * N, 1], f32)
            nc.vector.reduce_sum(out=ua[:], in_=usq[:], axis=X)
            # first ACT table-needing op: Sqrt (sqrt_and_others also covers
            # the Copy/Square used by the setup chain)
            nc.scalar.activation(out=F[0:2 * N, 2:3], in_=ua[:], func=Act.Sqrt,
                                 bias=eps_sb[:])

            # ================= hot loop =====================================
            # Per row tile: chunks 0,1 (gt cols [0:2048)) -> ACT softmin
            # (exp(-A d^2), sum accumulator); chunks 2,3 -> DVE exact min-
            # reduce.  Four 2-bank psum slots keep the PE from stalling.
            # Tile 6 is all-DVE so the deferred Ln (plus its activation-
            # table load) overlaps tile 6 instead of serializing at the end.
            pmin = singles.tile([128, 7], f32)
            esums = singles.tile([128, 7, 2], f32)
            nc.vector.memset(esums[:], 0.0)
            rmins = singles.tile([128, 7, 2], f32)
            esb = singles.tile([128, 1024], f32)
            t6x = singles.tile([128, 2], f32)
            i3 = singles.tile([128, 2], f32)
            for t in range(7):
                for q in range(4):
                    ph = psum.tile([128, 1024], f32, tag="hot", name="ph")
                    for j in range(2):
                        co = 1024 * q + 512 * j
                        nc.tensor.matmul(
                            ph[:, 512 * j:512 * (j + 1)],
                            kaugT[:, 128 * t:128 * (t + 1)],
                            gaug[:, co:co + 512],
                            start=True, stop=True)
                    if q < 2 and t < 6:
                        nc.scalar.activation(out=esb[:], in_=ph[:],
                                             func=Act.Exp, scale=-A,
                                             accum_out=esums[:, t, q:q + 1])
                    elif q < 2:
                        nc.vector.tensor_reduce(
                            out=t6x[:, q:q + 1], in_=ph[:], axis=X, op=Alu.min)
                    else:
                        nc.vector.tensor_reduce(
                            out=rmins[:, t, q - 2:q - 1], in_=ph[:],
                            axis=X, op=Alu.min)
            esum7 = singles.tile([128, 7], f32)
            nc.vector.reduce_sum(out=esum7[:], in_=esums[:], axis=X)
            rmin7 = singles.tile([128, 7], f32)
            nc.vector.tensor_reduce(out=rmin7[:], in_=rmins[:], axis=X,
                                    op=Alu.min)
            # tile 6 ran all-exact: fold its extra chunk pair in
            nc.vector.tensor_tensor(out=t6x[:, 0:1], in0=t6x[:, 0:1],
                                    in1=t6x[:, 1:2], op=Alu.min)
            nc.vector.tensor_tensor(out=rmin7[:, 6:7], in0=rmin7[:, 6:7],
                                    in1=t6x[:, 0:1], op=Alu.min)
            lnv7 = singles.tile([128, 7], f32)
            nc.scalar.activation(out=lnv7[:], in_=esum7[:], func=Act.Ln)
            nc.vector.scalar_tensor_tensor(
                out=pmin[:], in0=lnv7[:], scalar=-1.0 / A,
                in1=rmin7[:], op0=Alu.mult, op1=Alu.min)
            nc.vector.tensor_copy(out=F[:, 3:4], in_=pmin[:, 0:1])
            nc.vector.reduce_sum(out=i3[:, 0:1], in_=pmin[:, 1:4], axis=X)
            nc.vector.reduce_sum(out=i3[:, 1:2], in_=pmin[:, 4:7], axis=X)

            # ================= tail =========================================
            cmall = singles.tile([128, 6], f32)
            nc.vector.memset(cmall[:], 0.0)
            for g, (g0, cnt) in enumerate(GROUPS):
                sel1 = psum.tile([cnt, 4], f32, tag="hot", name="sel1")
                nc.tensor.matmul(sel1[:], s12_sb[:, g0:g0 + cnt], F[:],
                                 start=True, stop=True)
                sel2 = psum.tile([cnt, 4], f32, tag="hot", name="sel2")
                nc.tensor.matmul(sel2[:], s12_sb[:, PAIR2 + g0:PAIR2 + g0 + cnt],
                                 F[:], start=True, stop=True)
                s1b = work.tile([cnt, 4], f32)
                nc.scalar.copy(out=s1b[:], in_=sel1[:])
                cd = work.tile([cnt, 1], f32)
                if g < 2:
                    nc.vector.scalar_tensor_tensor(
                        out=cd[:], in0=sel2[:, 3:4], scalar=i3[0:cnt, g:g + 1],
                        in1=s1b[:, 3:4], op0=Alu.add, op1=Alu.add)
                else:
                    nc.vector.tensor_add(cd[:], sel2[:, 3:4], s1b[:, 3:4])
                nc.vector.tensor_single_scalar(
                    out=cmall[0:cnt, 2 * g + 1:2 * g + 2], in_=cd[:],
                    scalar=float(COUNT * MAXDIS), op=Alu.is_lt)
                prod = work.tile([cnt, 3], f32)
                nc.vector.tensor_mul(prod[:], sel2[:, 0:3], s1b[:, 0:3])
                dt0 = work.tile([cnt, 1], f32)
                nc.vector.tensor_add(dt0[:], prod[:, 0:1], prod[:, 1:2])
                nc.vector.tensor_reduce(out=dt0[:], in_=dt0[:], axis=X,
                                        op=Alu.max, apply_absolute_value=True)
                rc = work.tile([cnt, 1], f32)
                nc.vector.reciprocal(out=rc[:], in_=prod[:, 2:3])
                nc.vector.scalar_tensor_tensor(
                    out=cmall[0:cnt, 2 * g:2 * g + 1], in0=dt0[:], scalar=rc[:],
                    in1=cmall[0:cnt, 2 * g + 1:2 * g + 2],
                    op0=Alu.mult, op1=Alu.mult)

            tot = psum.tile([1, 6], f32, tag="hot", name="tot")
            nc.tensor.matmul(tot[:], ones_sb[:], cmall[:], start=True,
                             stop=True)
            tot_sb = work.tile([1, 6], f32)
            nc.scalar.copy(out=tot_sb[:], in_=tot[:])
            acc = singles.tile([1, 2], f32)
            nc.vector.tensor_add(acc[:], tot_sb[:, 0:2], tot_sb[:, 2:4])
            nc.vector.tensor_add(acc[:], acc[:], tot_sb[:, 4:6])
            nc.sync.dma_start(out=out_d.rearrange("(a b) -> a b", a=1),
                              in_=acc[:])

    nc.compile()
    return nc


def _in_maps(recon_points, gt_points):
    ct, s12, wt = _get_consts()
    recon_points = np.ascontiguousarray(recon_points, np.float32)
    gt_points = np.ascontiguousarray(gt_points, np.float32)
    maps = []
    for k in range(N_CORES):
        maps.append({
            "recon": recon_points[BPC * k:BPC * (k + 1)],
            "gt": gt_points[BPC * k:BPC * (k + 1)],
            "ct": ct, "s12": s12, "wt": wt,
        })
    return maps


def kernel(recon_points: np.ndarray, gt_points: np.ndarray) -> np.ndarray:
    from concourse.bass_utils import run_bass_kernel_spmd

    global _COMPILED
    if _COMPILED is None:
        _COMPILED = _build()
    nc = _COMPILED

    res = run_bass_kernel_spmd(nc, _in_maps(recon_points, gt_points),
                               core_ids=list(range(N_CORES)))
    partials = np.stack([r["out"] for r in res.results])  # [8, 2]
    cos_sum = partials[:, 0].sum(dtype=np.float32)
    cnt = partials[:, 1].sum(dtype=np.float32)
    return np.float32(cos_sum / (np.float32(1.0) + cnt))
